# revision 33
# baseline (speedup 1.0000x reference)
"""Trainium2 Bass kernel for a dense transformer block (nn_Block_71949292143252).

Reference computation (B=4, T=2048, D=1024, H=16, HS=64):
    h  = LN1(x);  q,k,v = h @ Wq/Wk/Wv (per head)
    attn = causal-softmax(q k^T / sqrt(HS)) @ v        (concat heads)
    x1 = x + attn @ Wproj + bproj
    out = x1 + relu(LN2(x1) @ W1 + b1) @ W2 + b2

Sharding over 8 NeuronCores: core r handles batch r//2 and head-group r%2
(8 of 16 heads).  Attention is head-split over the full sequence; the
post-attention projection partials are summed across each core pair with
TWO ReduceScatters (pair shares one batch element) over interleaved
128-row tiles: RS_A (tiles 0-7) fires after tq chunks 0-1 so that the
first half of LN2+FFN runs as filler work inside attention chunks 2-3,
soaking up the idle PE in that chain-bound phase; RS_B + the second FFN
half form the tail.  Each core owns every other 128-row tile.

Everything on-device runs in "transposed" layout [feature, token] so that
no transposes are ever needed:
  - LN stats = ones-vector matmuls (partition reduction on PE)
  - Q^T,K^T = W-stationary matmuls over Z^T
  - scores computed as S^T = K Q^T tiles [tk, tq]; exp on ACT (no max
    subtraction -- scores here are bounded far below exp overflow); P^T
    feeds the P@V matmul directly with V~=[V|1] stationary (the ones
    column accumulates the softmax denominator)
  - FFN runs on Z2^T with W1/W2 natural layout, so b1/b2/bproj are
    per-partition ACT biases.
"""

import contextlib
import os
import sys

for _p in ("/opt/trn_rl_repo", "/root/.axon_site/_ro/trn_rl_repo"):
    if os.path.isdir(_p) and _p not in sys.path:
        sys.path.insert(0, _p)

import numpy as np
import ml_dtypes

import concourse.bacc as bacc
import concourse.mybir as mybir
import concourse.tile as tile
from concourse.bass_utils import run_bass_kernel_spmd

BF16 = ml_dtypes.bfloat16

# Problem shape (hardcoded per spec).
B, T, D, H, HS = 4, 2048, 1024, 16, 64
EPS = 1e-5
NCORES = 8
HPC = H // 2          # heads per core = 8
NPAIR = HPC // 2      # head pairs per core = 4
TQ = 512              # tq chunk width (one fp32 PSUM bank)
NTQC = T // TQ        # 4
NTK = T // 128        # 16
KD = D // 128         # 8
THALF = T // 2        # 1024 rows per core after the reduce-scatter
NFF = 4 * D           # 4096
NNT = NFF // 128      # 32
DT_F32 = mybir.dt.float32
DT_BF16 = mybir.dt.bfloat16

_NC_CACHE = {}
TRACE = False
LAST_RESULTS = None


def _build_program():
    nc = bacc.Bacc(
        "TRN2",
        target_bir_lowering=False,
        debug=False,
        enable_asserts=False,
        num_devices=NCORES,
    )

    io = {}
    io["xb_d"] = nc.declare_dram_parameter("xb", [128, KD * T], DT_BF16, isOutput=False)
    io["xown_d"] = nc.declare_dram_parameter("xown", [128, KD * THALF], DT_F32, isOutput=False)
    io["wq_d"] = nc.declare_dram_parameter("wq", [128, KD * NPAIR * 128], DT_BF16, isOutput=False)
    io["wk_d"] = nc.declare_dram_parameter("wk", [128, KD * NPAIR * 128], DT_BF16, isOutput=False)
    io["wv_d"] = nc.declare_dram_parameter("wv", [128, KD * 512], DT_BF16, isOutput=False)
    io["wp_d"] = nc.declare_dram_parameter("wp", [128, 4 * KD * 128], DT_BF16, isOutput=False)
    io["w1_d"] = nc.declare_dram_parameter("w1", [D, NFF], DT_BF16, isOutput=False)
    io["w2_d"] = nc.declare_dram_parameter("w2", [NFF, D], DT_BF16, isOutput=False)
    io["masks_d"] = nc.declare_dram_parameter("masks", [128, 896], DT_BF16, isOutput=False)
    io["qb_d"] = nc.declare_dram_parameter("qb", [128, NPAIR], DT_F32, isOutput=False)
    io["kb_d"] = nc.declare_dram_parameter("kb", [128, NPAIR], DT_F32, isOutput=False)
    io["bproj_d"] = nc.declare_dram_parameter("bproj", [128, KD], DT_F32, isOutput=False)
    io["b1_d"] = nc.declare_dram_parameter("b1", [128, NNT], DT_F32, isOutput=False)
    io["b2_d"] = nc.declare_dram_parameter("b2", [128, KD], DT_F32, isOutput=False)
    io["out_d"] = nc.declare_dram_parameter("outT", [D, THALF], DT_F32, isOutput=True)

    with tile.TileContext(nc) as tc:
        _emit(nc, tc, io)
    nc.compile()
    return nc


def _ln_transposed(nc, tc, pS, psum_st, ones_pair, eps1, src, ncols, dst, psum_tag="pv"):
    """LayerNorm in transposed layout: dst[k] = (src[k]-mu)*rstd per column.

    src/dst: SBUF tiles [128, KD, ncols]; src bf16, dst bf16.
    Stats via ones-matmul partition reduction; per-chunk broadcasts.
    """
    AF = mybir.ActivationFunctionType
    f32, bf16 = DT_F32, DT_BF16
    ones1, ones1f = ones_pair
    ones_mean = ones1f if src.dtype == DT_F32 else ones1
    nchunk = ncols // TQ
    for c in range(nchunk):
        sqs = []
        for k in range(KD):
            sq = pS.tile([128, TQ], bf16, tag="sq", name=f"sq_{c}_{k}", bufs=3)
            nc.vector.tensor_mul(
                out=sq, in0=src[:, k, c * TQ : (c + 1) * TQ],
                in1=src[:, k, c * TQ : (c + 1) * TQ],
            )
            sqs.append(sq)
        # sequential stats: hold only one shared-psum slot at a time
        ps_mean = psum_st.tile([1, TQ], f32, tag=psum_tag, name=f"ps_mean_{c}")
        for k in range(KD):
            nc.tensor.matmul(
                out=ps_mean, lhsT=ones_mean, rhs=src[:, k, c * TQ : (c + 1) * TQ],
                start=(k == 0), stop=(k == KD - 1),
            )
        mu = pS.tile([1, TQ], f32, tag="r1", bufs=1, name=f"mu_{c}")
        nc.vector.tensor_scalar_mul(out=mu, in0=ps_mean, scalar1=1.0 / D)
        ps_msq = psum_st.tile([1, TQ], f32, tag=psum_tag, name=f"ps_msq_{c}")
        for k in range(KD):
            nc.tensor.matmul(
                out=ps_msq, lhsT=ones1, rhs=sqs[k],
                start=(k == 0), stop=(k == KD - 1),
            )
        var = pS.tile([1, TQ], f32, tag="r2", bufs=1, name=f"var_{c}")
        nc.vector.tensor_scalar_mul(out=var, in0=ps_msq, scalar1=1.0 / D)
        musq = pS.tile([1, TQ], f32, tag="r3", bufs=1, name=f"musq_{c}")
        nc.vector.tensor_mul(out=musq, in0=mu, in1=mu)
        nc.vector.tensor_sub(out=var, in0=var, in1=musq)
        nc.scalar.activation(out=var, in_=var, func=AF.Sqrt, bias=eps1, scale=1.0)
        arow = pS.tile([1, TQ], bf16, tag="r4", bufs=1, name=f"arow_{c}")
        rstd = pS.tile([1, TQ], f32, tag="r5", bufs=1, name=f"rstd_{c}")
        nc.vector.reciprocal(out=rstd, in_=var)
        nc.vector.tensor_copy(out=arow, in_=rstd)
        brow = pS.tile([1, TQ], bf16, tag="r6", bufs=1, name=f"brow_{c}")
        negmu = pS.tile([1, TQ], f32, tag="r7", bufs=1, name=f"negmu_{c}")
        nc.vector.tensor_mul(out=negmu, in0=mu, in1=rstd)
        nc.vector.tensor_scalar_mul(out=negmu, in0=negmu, scalar1=-1.0)
        nc.vector.tensor_copy(out=brow, in_=negmu)
        a_b = pS.tile([128, TQ], bf16, tag="lnab", name=f"a_b_{c}", bufs=2)
        b_b = pS.tile([128, TQ], bf16, tag="lnbb", name=f"b_b_{c}", bufs=2)
        nc.gpsimd.partition_broadcast(a_b, arow, channels=128)
        nc.gpsimd.partition_broadcast(b_b, brow, channels=128)
        for k in range(KD):
            zk = pS.tile([128, TQ], bf16, tag="ztmp", name=f"ztmp_{c}_{k}", bufs=2)
            nc.vector.tensor_mul(out=zk, in0=src[:, k, c * TQ : (c + 1) * TQ], in1=a_b)
            nc.vector.tensor_add(out=dst[:, k, c * TQ : (c + 1) * TQ], in0=zk, in1=b_b)


def _emit(nc, tc, io):
    f32, bf16 = DT_F32, DT_BF16
    AF = mybir.ActivationFunctionType
    ALU = mybir.AluOpType

    ctx = contextlib.ExitStack()
    with ctx:
        # ---------------- pools ----------------
        pA = ctx.enter_context(tc.tile_pool(name="pA", bufs=1))   # xb -> aT chunks
        pB = ctx.enter_context(tc.tile_pool(name="pB", bufs=1))   # zT -> xown/x1T
        pQ = ctx.enter_context(tc.tile_pool(name="pQ", bufs=1))   # qT -> z2T
        pK = ctx.enter_context(tc.tile_pool(name="pK", bufs=1))   # kT -> x1b
        pV = ctx.enter_context(tc.tile_pool(name="pV", bufs=1))   # v~ -> rs result
        pW = ctx.enter_context(tc.tile_pool(name="pW", bufs=1))   # weights/masks/biases
        pS = ctx.enter_context(tc.tile_pool(name="pS", bufs=2))   # small transients
        pP = ctx.enter_context(tc.tile_pool(name="pP", bufs=3))   # P^T tiles, copybacks
        pStream = ctx.enter_context(tc.tile_pool(name="pStream", bufs=2))  # w1/w2 stream
        dram = ctx.enter_context(tc.tile_pool(name="dram", bufs=1, space="DRAM"))
        psum_mm = ctx.enter_context(tc.tile_pool(name="psum_mm", bufs=2, space="PSUM"))
        psum_pv = ctx.enter_context(tc.tile_pool(name="psum_pv", bufs=2, space="PSUM"))
        psum_sc = ctx.enter_context(tc.tile_pool(name="psum_sc", bufs=2, space="PSUM"))

        # ---------------- persistent inputs ----------------
        xb = pA.tile([128, KD, T], bf16, tag="bigA", name="xb")
        xbr = io["xb_d"][:, :].rearrange("p (k t) -> p k t", k=KD)
        for c in range(NTQC):
            for k in range(KD):
                nc.sync.dma_start(
                    out=xb[:, k, c * TQ : (c + 1) * TQ],
                    in_=xbr[:, k, c * TQ : (c + 1) * TQ],
                )
        wq = pW.tile([128, KD * NPAIR * 128], bf16, name="wq")
        nc.sync.dma_start(out=wq, in_=io["wq_d"][:, :])
        wk = pW.tile([128, KD * NPAIR * 128], bf16, name="wk")
        nc.sync.dma_start(out=wk, in_=io["wk_d"][:, :])
        wv = pW.tile([128, KD, 512], bf16, name="wv")
        nc.sync.dma_start(out=wv, in_=io["wv_d"][:, :].rearrange("p (k n) -> p k n", k=KD))
        wp = pW.tile([128, 4 * KD * 128], bf16, name="wp")
        nc.sync.dma_start(out=wp, in_=io["wp_d"][:, :])
        masks = pW.tile([128, 896], bf16, name="masks")
        nc.sync.dma_start(out=masks, in_=io["masks_d"][:, :])
        qb = pW.tile([128, NPAIR], f32, name="qb")
        nc.sync.dma_start(out=qb, in_=io["qb_d"][:, :])
        kb = pW.tile([128, NPAIR], f32, name="kb")
        nc.sync.dma_start(out=kb, in_=io["kb_d"][:, :])
        bproj = pW.tile([128, KD], f32, name="bproj")
        nc.sync.dma_start(out=bproj, in_=io["bproj_d"][:, :])
        b1 = pW.tile([128, NNT], f32, name="b1")
        nc.sync.dma_start(out=b1, in_=io["b1_d"][:, :])
        b2 = pW.tile([128, KD], f32, name="b2")
        nc.sync.dma_start(out=b2, in_=io["b2_d"][:, :])

        ones1 = pW.tile([128, 1], bf16, name="ones1")
        nc.vector.memset(ones1, 1.0)
        ones1f = pW.tile([128, 1], f32, name="ones1f")
        nc.vector.memset(ones1f, 1.0)
        eps1 = pW.tile([1, 1], f32, name="eps1")
        nc.vector.memset(eps1, EPS)

        # ---------------- LN1 ----------------
        zT = pB.tile([128, KD, T], bf16, tag="bigB", name="zT")
        _ln_transposed(nc, tc, pS, psum_pv, (ones1, ones1f), eps1, xb, T, zT)

        # ---------------- QKV ----------------
        qT = pQ.tile([128, NPAIR, T], bf16, tag="bigQ", name="qT")
        kT = pK.tile([128, NPAIR, T], bf16, tag="bigK", name="kT")
        vt = pV.tile([128, NTK, HPC, 65], bf16, tag="bigV", name="vt")
        nc.vector.memset(vt[:, :, :, 64:65], 1.0)

        def q_unit(p, c):
            ps_q = psum_mm.tile([128, TQ], f32, tag="mm", name=f"ps_q_{p}_{c}")
            for k in range(KD):
                nc.tensor.matmul(
                    out=ps_q,
                    lhsT=wq[:, (k * NPAIR + p) * 128 : (k * NPAIR + p + 1) * 128],
                    rhs=zT[:, k, c * TQ : (c + 1) * TQ],
                    start=(k == 0), stop=(k == KD - 1),
                )
            nc.scalar.activation(
                out=qT[:, p, c * TQ : (c + 1) * TQ], in_=ps_q,
                func=AF.Identity, bias=qb[:, p : p + 1], scale=1.0,
            )

        def k_unit(p, c):
            ps_k = psum_mm.tile([128, TQ], f32, tag="mm", name=f"ps_k_{p}_{c}")
            for k in range(KD):
                nc.tensor.matmul(
                    out=ps_k,
                    lhsT=wk[:, (k * NPAIR + p) * 128 : (k * NPAIR + p + 1) * 128],
                    rhs=zT[:, k, c * TQ : (c + 1) * TQ],
                    start=(k == 0), stop=(k == KD - 1),
                )
            nc.vector.tensor_scalar_add(
                out=kT[:, p, c * TQ : (c + 1) * TQ], in0=ps_k,
                scalar1=kb[:, p : p + 1],
            )

        def v_unit(i):
            ps_v = psum_mm.tile([128, 512], f32, tag="mm", name=f"ps_v_{i}")
            for k in range(KD):
                nc.tensor.matmul(
                    out=ps_v, lhsT=zT[:, k, i * 128 : (i + 1) * 128],
                    rhs=wv[:, k, :],
                    start=(k == 0), stop=(k == KD - 1),
                )
            nc.scalar.activation(
                out=vt[:, i, :, 0:64],
                in_=ps_v.rearrange("p (h s) -> p h s", h=HPC),
                func=AF.Copy,
            )

        # inline: everything attention chunks 0,1 need (Q/K chunks 0-1,
        # V tiles 0-7); the rest is deferred into chunk-0/1 pair boundaries
        for p in range(NPAIR):
            for c in range(2):
                q_unit(p, c)
                k_unit(p, c)
        for i in range(8):
            v_unit(i)

        def qkv_deferred():
            for c in range(2, NTQC):
                for p in range(NPAIR):
                    k_unit(p, c)
                    yield
                    q_unit(p, c)
                    yield
                for i in range(4 * c, 4 * c + 4):
                    v_unit(i)
                    yield

        # ---------------- attention + projection + interleaved FFN ----------
        # Row ownership interleaved at 128-row tiles (rank r%2==0 owns even
        # tiles).  Two ReduceScatters: A = tiles 0-7 (after chunks 0,1),
        # B = tiles 8-15 (after chunks 2,3).  FFN pass A runs as filler work
        # inside attention chunks 2,3 to soak up idle PE there.
        cc_inA = dram.tile([2, D, 512], bf16, name="cc_inA")
        cc_outA = dram.tile([D, 512], bf16, name="cc_outA")
        cc_inB = dram.tile([2, D, 512], bf16, name="cc_inB")
        cc_outB = dram.tile([D, 512], bf16, name="cc_outB")

        xown = pB.tile([128, KD, THALF], f32, tag="bigB", name="xown")
        x1T = xown
        w1r = io["w1_d"][:, :].rearrange("(k p) n -> p k n", p=128)
        w2r = io["w2_d"][:, :].rearrange("(k p) d -> p k d", p=128)

        def emit_rs(pp):
            nc.gpsimd.collective_compute(
                "ReduceScatter",
                ALU.add,
                replica_groups=[[0, 1], [2, 3], [4, 5], [6, 7]],
                ins=[(cc_inA if pp == 0 else cc_inB)[:, :, :]],
                outs=[(cc_outA if pp == 0 else cc_outB)[:, :]],
            )

        def make_pass(pp):
            col = pp * 512
            cc_out = cc_outA if pp == 0 else cc_outB
            rs = pW.tile([128, KD, 512], bf16, tag="wq" if pp == 0 else "wk",
                         name=f"rs_{pp}")
            nc.sync.dma_start(
                out=rs, in_=cc_out[:, :].rearrange("(k p) t -> p k t", p=128)
            )
            for k in range(KD):
                nc.vector.tensor_add(
                    out=x1T[:, k, col : col + 512],
                    in0=xown[:, k, col : col + 512], in1=rs[:, k, :],
                )
                nc.scalar.activation(
                    out=x1T[:, k, col : col + 512], in_=x1T[:, k, col : col + 512],
                    func=AF.Identity, bias=bproj[:, k : k + 1], scale=1.0,
                )
            yield
            z2T = pW.tile([128, KD, 512], bf16, tag="wv", name=f"z2T_{pp}")
            _ln_transposed(
                nc, tc, pS, psum_mm, (ones1, ones1f), eps1,
                x1T[:, :, col : col + 512], 512, z2T, psum_tag="mm",
            )
            yield
            aT = pA.tile([128, NNT, TQ], bf16, tag="bigA", name=f"aT_{pp}")
            for n in range(NNT):
                w1c = pStream.tile([128, KD, 128], bf16, tag="w1c",
                                   name=f"w1c_{pp}_{n}", bufs=3)
                nc.sync.dma_start(out=w1c, in_=w1r[:, :, n * 128 : (n + 1) * 128])
                ps_f = psum_mm.tile([128, TQ], f32, tag="mm", name=f"ps_f_{pp}_{n}")
                for k in range(KD):
                    nc.tensor.matmul(
                        out=ps_f, lhsT=w1c[:, k, :], rhs=z2T[:, k, :],
                        start=(k == 0), stop=(k == KD - 1),
                    )
                nc.scalar.activation(
                    out=aT[:, n, :], in_=ps_f,
                    func=AF.Relu, bias=b1[:, n : n + 1], scale=1.0,
                )
                if n % 2 == 1:
                    yield
            for dt in range(KD):
                ps_o = psum_mm.tile([128, TQ], f32, tag="mm", name=f"ps_o_{pp}_{dt}")
                for hh in range(2):
                    w2c = pStream.tile([128, NNT // 2, 128], bf16, tag="w2c",
                                       name=f"w2c_{pp}_{dt}_{hh}", bufs=3)
                    nc.sync.dma_start(
                        out=w2c,
                        in_=w2r[:, hh * (NNT // 2) : (hh + 1) * (NNT // 2),
                                dt * 128 : (dt + 1) * 128],
                    )
                    for kk in range(NNT // 2):
                        k2 = hh * (NNT // 2) + kk
                        nc.tensor.matmul(
                            out=ps_o, lhsT=w2c[:, kk, :], rhs=aT[:, k2, :],
                            start=(k2 == 0), stop=(k2 == NNT - 1),
                        )
                ostg = pP.tile([128, TQ], f32, tag="ostg", name=f"ostg_{pp}_{dt}", bufs=2)
                nc.vector.tensor_add(
                    out=ostg, in0=ps_o, in1=x1T[:, dt, col : col + 512]
                )
                nc.scalar.activation(
                    out=ostg, in_=ostg,
                    func=AF.Identity, bias=b2[:, dt : dt + 1], scale=1.0,
                )
                nc.sync.dma_start(
                    out=io["out_d"][dt * 128 : (dt + 1) * 128, col : col + 512],
                    in_=ostg,
                )
                yield

        passA = None
        qkvD = qkv_deferred()

        for c in range(NTQC):
            ni = 4 * c + 4
            attnT = [
                pS.tile([128, TQ], bf16, tag=f"attnT{k2}", name=f"attnT_{c}_{k2}", bufs=1)
                for k2 in range(NPAIR)
            ]
            for p in range(NPAIR):
                pv = [
                    psum_pv.tile([128, TQ], f32, tag="pv", name=f"pv_{c}_{p}_{h}")
                    for h in range(2)
                ]
                def geom(i):
                    # diagonal blocks (i-4c = o >= 0): columns < 128*o are
                    # fully masked -- trim them from the matmuls/exp/mask.
                    o = i - 4 * c
                    cut = 128 * o if o > 0 else 0
                    return o, cut, TQ - cut

                def emit_scores_exp(i):
                    o, cut, w = geom(i)
                    # both heads' scores in one 2-bank psum tile -> single exp
                    s_ps = psum_sc.tile(
                        [128, 2, TQ], f32, tag="sc", name=f"s_{c}_{p}_{i}"
                    )
                    for h in range(2):
                        nc.tensor.matmul(
                            out=s_ps[:, h, :w],
                            lhsT=kT[64 * h : 64 * h + 64, p, i * 128 : (i + 1) * 128],
                            rhs=qT[64 * h : 64 * h + 64, p, c * TQ + cut : (c + 1) * TQ],
                            start=True, stop=True,
                        )
                    pt = pP.tile([128, 2, TQ], bf16, tag="pt", name=f"pt_{c}_{p}_{i}", bufs=4)
                    nc.scalar.activation(
                        out=pt[:, :, :w], in_=s_ps[:, :, :w], func=AF.Exp,
                        scale=1.0 / np.sqrt(HS),
                    )
                    if o >= 0:
                        for h in range(2):
                            nc.gpsimd.tensor_mul(
                                out=pt[:, h, :w], in0=pt[:, h, :w],
                                in1=masks[:, 384 : 384 + w],
                            )
                    return pt

                def emit_pv(i, pt):
                    o, cut, w = geom(i)
                    for h in range(2):
                        nc.tensor.matmul(
                            out=pv[h][0:65, cut:TQ],
                            lhsT=vt[:, i, 2 * p + h, :],
                            rhs=pt[:, h, :w],
                            start=(i == 0), stop=(i == ni - 1),
                            skip_group_check=True,
                        )

                # software pipeline: scores/exp run one iteration ahead of PV
                stage = {0: emit_scores_exp(0)}
                for i in range(ni):
                    if i + 1 < ni:
                        stage[i + 1] = emit_scores_exp(i + 1)
                    emit_pv(i, stage.pop(i))
                for h in range(2):
                    rrow = pS.tile([1, TQ], bf16, tag="rrow", bufs=1, name=f"rr_{c}_{p}_{h}")
                    with nc.allow_low_precision(reason="softmax recip in bf16"):
                        nc.vector.reciprocal(out=rrow, in_=pv[h][64:65, :])
                    rb = pP.tile([64, TQ], bf16, tag="rb", bufs=2, name=f"rb_{c}_{p}_{h}")
                    nc.gpsimd.partition_broadcast(rb, rrow, channels=64)
                    nc.vector.tensor_mul(
                        out=attnT[p][64 * h : 64 * h + 64, :],
                        in0=pv[h][0:64, :], in1=rb,
                    )
                if qkvD is not None:
                    for _ in range(3):
                        try:
                            next(qkvD)
                        except StopIteration:
                            qkvD = None
                            break
                if passA is not None:
                    for _ in range(2):
                        try:
                            next(passA)
                        except StopIteration:
                            passA = None
                            break
            for dt in range(KD):
                ps_p = psum_mm.tile([128, TQ], f32, tag="mm", name=f"ps_p_{c}_{dt}")
                for k2 in range(NPAIR):
                    nc.tensor.matmul(
                        out=ps_p,
                        lhsT=wp[:, (k2 * KD + dt) * 128 : (k2 * KD + dt + 1) * 128],
                        rhs=attnT[k2],
                        start=(k2 == 0), stop=(k2 == NPAIR - 1),
                    )
                stg = pP.tile([128, TQ], bf16, tag="stg", name=f"stg_{c}_{dt}", bufs=2)
                nc.scalar.activation(out=stg, in_=ps_p, func=AF.Copy)
                # chunk c = global tiles 4c..4c+3; even -> shard 0, odd -> 1
                cc_in = cc_inA if c < 2 else cc_inB
                so = (c % 2) * 256
                sg = stg[:, :].rearrange("p (j f) -> p j f", j=4)
                for par in range(2):
                    nc.sync.dma_start(
                        out=cc_in[par, dt * 128 : (dt + 1) * 128, so : so + 256]
                        .rearrange("p (j f) -> p j f", j=2),
                        in_=sg[:, par::2, :],
                    )
            if c == 1:
                while qkvD is not None:
                    try:
                        next(qkvD)
                    except StopIteration:
                        qkvD = None
                nc.sync.dma_start(
                    out=xown,
                    in_=io["xown_d"][:, :].rearrange("p (k t) -> p k t", k=KD),
                )
                emit_rs(0)
                passA = make_pass(0)
            if c == 3:
                emit_rs(1)

        while passA is not None:
            try:
                next(passA)
            except StopIteration:
                passA = None

        for _ in make_pass(1):
            pass


def _get_nc():
    if "nc" not in _NC_CACHE:
        _NC_CACHE["nc"] = _build_program()
    return _NC_CACHE["nc"]


def _prep_inputs(x, Wq, Wk, Wv, Wproj, bproj, ln1_g, ln1_b, ln2_g, ln2_b, W1, b1, W2, b2):
    """Build the 8 per-core input dicts (host-side sharding + layout prep)."""
    f32 = np.float32
    x = np.asarray(x, f32)
    Wq, Wk, Wv = np.asarray(Wq, f32), np.asarray(Wk, f32), np.asarray(Wv, f32)
    Wproj = np.asarray(Wproj, f32)
    W1, W2 = np.asarray(W1, f32), np.asarray(W2, f32)
    ln1_g, ln1_b = np.asarray(ln1_g, f32), np.asarray(ln1_b, f32)
    ln2_g, ln2_b = np.asarray(ln2_g, f32), np.asarray(ln2_b, f32)
    b1v, b2v, bpv = np.asarray(b1, f32), np.asarray(b2, f32), np.asarray(bproj, f32)

    # fold LN gains into weights; LN biases become additive bias projections
    Wq_e = ln1_g[None, :, None] * Wq      # [H, D, HS]
    Wk_e = ln1_g[None, :, None] * Wk
    Wv_e = ln1_g[None, :, None] * Wv
    qbias = np.einsum("d,hdk->hk", ln1_b, Wq_e)   # [H, HS]
    kbias = np.einsum("d,hdk->hk", ln1_b, Wk_e)
    vbias = np.einsum("d,hdk->hk", ln1_b, Wv_e)
    assert np.abs(vbias).max() < 1e-6, "nonzero ln1_b@Wv not supported"
    W1_e = ln2_g[:, None] * W1
    b1_e = b1v + ln2_b @ W1_e

    w1_h = np.ascontiguousarray(W1_e.astype(BF16))
    w2_h = np.ascontiguousarray(W2.astype(BF16))

    # wide causal mask for S^T tiles: W[p, g] = 1 iff g >= p + 384
    # (slice cols [384-128*o : 896-128*o] gives the offset-o diagonal mask)
    pp, gg = np.arange(128)[:, None], np.arange(896)[None, :]
    masks_h = np.ascontiguousarray((gg >= pp + 384).astype(BF16))

    def tile_cols(w):  # [D, M] -> [128, KD*M]: d-tile k at cols [k*M, (k+1)*M)
        Dd, M = w.shape
        return np.ascontiguousarray(
            w.reshape(KD, 128, M).transpose(1, 0, 2).reshape(128, KD * M)
        )

    in_maps = []
    for r in range(NCORES):
        b, t = r // 2, r % 2
        hbase = HPC * t
        heads = list(range(hbase, hbase + HPC))

        xT = np.ascontiguousarray(x[b].T)                     # [D, T] f32
        xb_h = np.ascontiguousarray(
            xT.astype(BF16).reshape(KD, 128, T).transpose(1, 0, 2).reshape(128, KD * T)
        )
        # interleaved ownership: core r%2==t owns global 128-row tiles
        # {t, t+2, ..., t+14}, packed in that order
        own_cols = np.concatenate(
            [np.arange((2 * j + t) * 128, (2 * j + t + 1) * 128) for j in range(KD)]
        )
        xown_h = np.ascontiguousarray(
            xT[:, own_cols]
            .reshape(KD, 128, THALF).transpose(1, 0, 2).reshape(128, KD * THALF)
        )

        def qk_layout(W_e):
            wpair = np.stack(
                [
                    np.concatenate([W_e[heads[2 * p]], W_e[heads[2 * p + 1]]], axis=1)
                    for p in range(NPAIR)
                ],
                axis=1,
            )  # [D, NPAIR, 128]
            w = wpair.reshape(KD, 128, NPAIR, 128).transpose(1, 0, 2, 3)
            return np.ascontiguousarray(
                w.reshape(128, KD * NPAIR * 128).astype(BF16)
            )

        wq_h = qk_layout(Wq_e)
        wk_h = qk_layout(Wk_e)
        wv_loc = np.concatenate([Wv_e[h] for h in heads], axis=1)  # [D, 512]
        wv_h = tile_cols(wv_loc.astype(BF16))
        wp_loc = Wproj[hbase * HS : (hbase + HPC) * HS, :]  # [512, D]
        wp_h = np.ascontiguousarray(
            wp_loc.reshape(4, 128, KD, 128)
            .transpose(1, 0, 2, 3)
            .reshape(128, 4 * KD * 128)
            .astype(BF16)
        )

        def bias_pairs(bias):
            return np.ascontiguousarray(
                np.stack(
                    [
                        np.concatenate([bias[heads[2 * p]], bias[heads[2 * p + 1]]])
                        for p in range(NPAIR)
                    ],
                    axis=1,
                ).astype(f32)
            )  # [128, NPAIR]

        in_maps.append(
            {
                "xb": xb_h,
                "xown": xown_h,
                "wq": wq_h,
                "wk": wk_h,
                "wv": wv_h,
                "wp": wp_h,
                "w1": w1_h,
                "w2": w2_h,
                "masks": masks_h,
                "qb": bias_pairs(qbias),
                "kb": bias_pairs(kbias),
                "bproj": np.ascontiguousarray(bpv.reshape(KD, 128).T.astype(f32)),
                "b1": np.ascontiguousarray(b1_e.reshape(NNT, 128).T.astype(f32)),
                "b2": np.ascontiguousarray(b2v.reshape(KD, 128).T.astype(f32)),
            }
        )
    return in_maps


def kernel(**inputs):
    global LAST_RESULTS
    in_maps = _prep_inputs(**inputs)
    nc = _get_nc()
    res = run_bass_kernel_spmd(nc, in_maps, core_ids=list(range(NCORES)), trace=TRACE)
    LAST_RESULTS = res
    out = np.empty((B, T, D), np.float32)
    for r in range(NCORES):
        b, t = r // 2, r % 2
        oT = np.asarray(res.results[r]["outT"], np.float32)
        for j in range(KD):
            g = 2 * j + t
            out[b, g * 128 : (g + 1) * 128, :] = oT[:, j * 128 : (j + 1) * 128].T
    return out
